# revision 18
# baseline (speedup 1.0000x reference)
"""NetVLAD pooling kernel for Trainium2, data-parallel over batch across 8 cores.

Computation per batch b (reference semantics):
  y      = x @ W_red.T + b_red            # [m, 64]
  yn     = y / ||y||_row                  # L2 normalize rows
  logits = yn @ W_lin.T + b_lin           # [m, 8]
  a      = softmax(logits, axis=1)
  vlad   = a.T @ yn - centroids * a.sum(0)[:, None]
  out    = l2norm_global(l2norm_rows(vlad).flatten())

Device-side structure (v14):
  - x ships pre-transposed [b, C, m] fp8e4m3; DMA at 2048-column granularity
    on the SP queue (consts go on the gpsimd queue so x loads start first).
  - Main matmuls run fp8 Normal mode (FWL active: 128-col stationary, fp8);
    4 matmuls per 128-row subtile.  DoubleRow measured slower (FD=73 < 128
    disables the FWL win; 95.1us -> 83.2us switching to Normal).
  - No bias matmul.  With y0 = x @ W_red.T (un-biased):
      yz[m, 0:8]  = r2  = y0 @ W_lin.T
      yz[m, 8]    = ip2 = 2*IPS * y0 . b_red   (extra fused weight column)
      yz[m, 9:73] = y0
    ss = ||y0+b||^2 = sum(y0^2) + ip2/IPS + ||b||^2 (Ln bias), logits bias
    W_lin@b_red added per-batch, and the aggregated b_red contribution
    restored at finalize via the ones-column.
  - Per tile: ACT stages all 73 cols PSUM->SBUF (one op); DVE squares the
    staged bf16 y0 at 2x and row-reduces into ss64.
  - Per batch, the softmax chain (batched over subtiles) runs on the Pool
    engine (TT/TS ops, SBUF-only) + ACT (Ln/Exp) + DVE (reduce/recip),
    with bf16 intermediates so DVE TT ops hit 2x mode.
    exp(b_lin) per-k factor cancels in the per-cluster L2 normalization
    denominator; it's still applied (am) so the softmax denominator is right.
  - vlad[k, 0:64 | asum | s] accumulated in PSUM over 64 small matmuls
    (stationary atil [128, 8] bf16); finalize per batch with negated
    centroids folded so the chain is short.
  - The LAST batch's pass runs in two chunks: subtiles 0:32 early (Pool,
    overlapped with remaining DMA), 32:64 + finalize in the tail on DVE/ACT.
"""
import numpy as np
import ml_dtypes
from contextlib import ExitStack

import concourse.bass as bass
import concourse.tile as tile
import concourse.bass_isa as bass_isa
from concourse import bacc, mybir
from concourse._compat import with_exitstack
from concourse.bass_utils import run_bass_kernel_spmd

bf16 = ml_dtypes.bfloat16
F32 = mybir.dt.float32
BF16 = mybir.dt.bfloat16
FP8 = mybir.dt.float8e4
fp8 = ml_dtypes.float8_e4m3

N_CORES = 8
B, M, C = 32, 8192, 512
K, D = 8, 64
B_LOC = B // N_CORES          # 4 batches per core
M_TILE = 1024                 # compute tile (rows)
DMA_COLS = 2048               # DMA granularity (2KB descriptors)
SUB = M // 128                # 64 subtiles of 128 rows per batch
NCH = C // 128                # 4 contraction chunks
W = 73                        # fused output cols: [r2(8) | ip2(1) | y0(64)]
W2 = 75                       # ysb cols: [r2 | ip2 | y0 | n | 1]
IPS = 64.0                    # ip2 column pre-scale (keeps fp8 weights normal)

XT_BUFS = 4
SB_BUFS = 2

# calibration knob (used by calib.py only; kernel() always runs "full"):
#   "full"      — normal kernel
#   "dma_only"  — emit only the DMA loads (no compute)
#   "one_span"  — DMA only span 0 per batch, compute reads it repeatedly
MODE = "full"
POOL_TT = True       # softmax-chain TT ops on Pool (False -> DVE)
POOL_MEMSET = True   # ones-column memset on Pool (False -> DVE)
USE_TTR = False      # tensor_tensor_reduce compiles but wedges the exec unit
                     # on this runtime (NRT_EXEC_UNIT_UNRECOVERABLE) - keep off
CHUNK_LAST = True    # split last batch's pass (False -> single tail pass)


@with_exitstack
def _netvlad_kernel(ctx: ExitStack, tc: tile.TileContext, out_d, xt_d, wcat_d,
                    wlbc_d, wl16_d, cb_d):
    nc = tc.nc
    AF = mybir.ActivationFunctionType
    OP = mybir.AluOpType

    consts = ctx.enter_context(tc.tile_pool(name="consts", bufs=1))
    xt_pool = ctx.enter_context(tc.tile_pool(name="xt", bufs=XT_BUFS))
    sb = ctx.enter_context(tc.tile_pool(name="work", bufs=SB_BUFS))
    bsb = ctx.enter_context(tc.tile_pool(name="bwork", bufs=SB_BUFS))
    outp = ctx.enter_context(tc.tile_pool(name="outp", bufs=1))
    yz_pool = ctx.enter_context(tc.tile_pool(name="yz", bufs=3, space="PSUM"))
    vlad_pool = ctx.enter_context(tc.tile_pool(name="vlad", bufs=2, space="PSUM"))

    if MODE == "dma_only":
        for b in range(B_LOC):
            xt_b = xt_d[b].rearrange("(j p) m -> p j m", p=128)
            for m0 in range(0, M, DMA_COLS):
                xt2 = xt_pool.tile([128, NCH, DMA_COLS], FP8, tag="xt")
                nc.sync.dma_start(xt2[:], xt_b[:, :, m0:m0 + DMA_COLS])
        return

    # wcat on the SP queue (SWDGE mis-handles its 512 tiny descriptors);
    # other consts on the gpsimd queue so xt loads queue right behind wcat
    wcat = consts.tile([128, NCH, W], FP8)
    nc.sync.dma_start(wcat[:], wcat_d.rearrange("j p t -> p j t"))
    wlbc = consts.tile([128, 9], F32)     # [W_lin@b_red (8, unused) | ||b||^2]
    nc.gpsimd.dma_start(wlbc[:], wlbc_d[:])
    wl16 = consts.tile([128, 16], BF16)   # [W_lin@b_red (8) | exp(b_lin) (8)]
    nc.gpsimd.dma_start(wl16[:], wl16_d[:])
    cb = consts.tile([K, 2, D], F32)      # [-centroids | b_red broadcast]
    nc.gpsimd.dma_start(cb[:], cb_d[:])

    outsb = outp.tile([K, B_LOC, D], F32)

    def process_tile(xt2, h0, rows, ss64, ysb, sub0):
        """One compute tile of `rows` rows starting at col h0 of xt2."""
        sub = rows // 128
        yz = yz_pool.tile([128, sub, 128], F32, tag="yz")
        for s in range(sub):
            for j in range(4):
                nc.tensor.matmul(
                    yz[:, s, :W],
                    xt2[:, j, h0 + s * 128:h0 + (s + 1) * 128],
                    wcat[:, j, :],
                    start=(j == 0), stop=(j == 3),
                )
        # stage all 73 cols PSUM->SBUF in one ACT op
        nc.scalar.activation(ysb[:, sub0:sub0 + sub, 0:W], yz[:, :, 0:W],
                             AF.Copy)
        # DVE squares the staged bf16 y0 (2x mode) and row-reduces into ss64
        sqs = sb.tile([128, 8, D], BF16, tag="sqs")
        nc.vector.tensor_tensor(
            out=sqs[:, :sub, :], in0=ysb[:, sub0:sub0 + sub, 9:W],
            in1=ysb[:, sub0:sub0 + sub, 9:W], op=OP.mult)
        nc.vector.reduce_sum(ss64[:, sub0:sub0 + sub], sqs[:, :sub, :],
                             axis=mybir.AxisListType.X)

    def emit_pass(b, ysb, ss64, s0, s1, ve, vlad):
        """Softmax-weight chain + vlad aggregation for subtiles [s0, s1).

        ve: engine namespace for the TT/TS ops (nc.gpsimd when overlapped
        under later DMA, nc.vector for the tail chunk)."""
        ns = s1 - s0
        # ss = sum(y0^2) + ip2/IPS (+ ||b||^2 via Ln bias)
        # (TensorScalarPtr is not a valid Pool opcode -> always DVE)
        sst = bsb.tile([128, ns], F32, tag=f"sst{s0}")
        nc.vector.scalar_tensor_tensor(
            out=sst[:], in0=ysb[:, s0:s1, 8], scalar=float(1.0 / IPS),
            in1=ss64[:, s0:s1], op0=OP.mult, op1=OP.add)
        lss = bsb.tile([128, ns], F32, tag=f"lss{s0}")
        nc.scalar.activation(lss[:], sst[:], AF.Ln, bias=wlbc[:, 8:9])
        inv = bsb.tile([128, ns], BF16, tag=f"inv{s0}")
        nc.scalar.activation(inv[:], lss[:], AF.Exp, scale=-0.5)
        # n = ||y|| = exp(+.5 ln ss) -> ysb col 73 (off critical path)
        nc.scalar.activation(ysb[:, s0:s1, W:W + 1], lss[:].unsqueeze(2),
                             AF.Exp, scale=0.5)
        invb = inv[:].unsqueeze(2).broadcast_to([128, ns, K])
        # t64 = (r2 + W_lin@b_red) * inv  (bf16 throughout)
        t0 = bsb.tile([128, ns, K], BF16, tag=f"t0{s0}")
        ve.tensor_tensor(
            out=t0[:], in0=ysb[:, s0:s1, 0:K],
            in1=wl16[:, 0:8].unsqueeze(1).broadcast_to([128, ns, K]),
            op=OP.add)
        t64 = bsb.tile([128, ns, K], BF16, tag=f"t64{s0}")
        ve.tensor_tensor(out=t64[:], in0=t0[:], in1=invb, op=OP.mult)
        e64 = bsb.tile([128, ns, K], BF16, tag=f"e64{s0}")
        nc.scalar.activation(e64[:], t64[:], AF.Exp)
        am = bsb.tile([128, ns, K], BF16, tag=f"am{s0}")
        ve.tensor_tensor(
            out=am[:], in0=e64[:],
            in1=wl16[:, 8:16].unsqueeze(1).broadcast_to([128, ns, K]),
            op=OP.mult)
        rs = bsb.tile([128, ns], F32, tag=f"rs{s0}")
        nc.vector.reduce_sum(rs[:], am[:], axis=mybir.AxisListType.X)
        rr = bsb.tile([128, ns], F32, tag=f"rr{s0}")
        nc.vector.reciprocal(rr[:], rs[:])
        q = bsb.tile([128, ns], BF16, tag=f"q{s0}")
        nc.vector.tensor_tensor(out=q[:], in0=inv[:], in1=rr[:], op=OP.mult)
        atil = bsb.tile([128, ns, K], BF16, tag=f"atil{s0}")
        ve.tensor_tensor(out=atil[:], in0=am[:],
                         in1=q[:].unsqueeze(2).broadcast_to([128, ns, K]),
                         op=OP.mult)
        # vlad[k] = [sum_m atil*y0 | sum atil*n (=a.sum) | sum atil (=s_k)]
        for s in range(s0, s1):
            nc.tensor.matmul(
                vlad[:], atil[:, s - s0, :], ysb[:, s, 9:W2],
                start=(s == 0), stop=(s == SUB - 1),
            )

    def emit_finalize(b, vlad, ve):
        """v = vlad_y + b*s_k - cent*asum, intra-normalize, global normalize.
        (cb[:,0] holds NEGATED centroids so the chain is short.)"""
        vz = sb.tile([K, D + 2], F32, tag="vz")
        nc.scalar.activation(vz[:], vlad[:], AF.Copy)
        bs = sb.tile([K, D], F32, tag="bs")
        nc.vector.tensor_scalar_mul(bs[:], cb[:, 1, :], vz[:, D + 1:D + 2])
        w = sb.tile([K, D], F32, tag="w")
        nc.vector.scalar_tensor_tensor(
            out=w[:], in0=cb[:, 0, :], scalar=vz[:, D:D + 1],
            in1=bs[:], op0=OP.mult, op1=OP.add)
        v = sb.tile([K, D], F32, tag="v")
        ve.tensor_tensor(out=v[:], in0=vz[:, 0:D], in1=w[:], op=OP.add)
        sck = sb.tile([K, D], F32, tag="sck")
        ssk = sb.tile([K, 1], F32, tag="ssk")
        if USE_TTR:
            nc.vector.tensor_tensor_reduce(
                out=sck[:], in0=v[:], in1=v[:], scale=1.0, scalar=0.0,
                op0=OP.mult, op1=OP.add, accum_out=ssk[:])
        else:
            nc.vector.tensor_tensor(out=sck[:], in0=v[:], in1=v[:],
                                    op=OP.mult)
            nc.vector.reduce_sum(ssk[:], sck[:], axis=mybir.AxisListType.X)
        lk = sb.tile([K, 1], F32, tag="lk")
        nc.scalar.activation(lk[:], ssk[:], AF.Ln)
        invk = sb.tile([K, 1], F32, tag="invk")
        nc.scalar.activation(invk[:], lk[:], AF.Exp, scale=-0.5)
        # after intra-normalization each of the K rows has norm exactly 1,
        # so the global norm is sqrt(K); fold 1/sqrt(K) into the multiply
        nc.vector.tensor_scalar(
            out=outsb[:, b, :], in0=v[:], scalar1=invk[:],
            scalar2=float(1.0 / np.sqrt(K)), op0=OP.mult, op1=OP.mult)

    # software pipeline: batch b-1's deferred pass is emitted after batch b's
    # first DMA span so its serial chain overlaps batch b's main matmuls.
    # The last batch's pass is chunked: first half early (Pool), second half
    # + finalize in the tail (DVE).
    pending = None
    vlad_last = None
    last = B_LOC - 1
    for b in range(B_LOC):
        ysb = bsb.tile([128, SUB, W2], BF16, tag="ysb")
        ss64 = bsb.tile([128, SUB], F32, tag="ss64")
        # ones column (cheap; per-batch so every read has a tracked writer)
        me = nc.gpsimd if POOL_MEMSET else nc.vector
        me.memset(ysb[:, :, W + 1:W + 2], 1.0)
        xt_b = xt_d[b].rearrange("(j p) m -> p j m", p=128)

        # DMA spans; batch 0 starts finer so the pipeline fills early
        if b == 0 and MODE != "one_span":
            spans = [(0, 512), (512, 512), (1024, 1024), (2048, 2048),
                     (4096, 2048), (6144, 2048)]
        else:
            spans = [(m0, DMA_COLS) for m0 in range(0, M, DMA_COLS)]
        first_xt = None
        for si, (m0, cols) in enumerate(spans):
            if MODE == "one_span" and first_xt is not None:
                xt2 = first_xt
            else:
                xt2 = xt_pool.tile([128, NCH, cols], FP8, tag="xt")
                nc.sync.dma_start(xt2[:], xt_b[:, :, m0:m0 + cols])
                if first_xt is None:
                    first_xt = xt2
            for h0 in range(0, cols, M_TILE):
                rows = min(M_TILE, cols - h0)
                process_tile(xt2, h0, rows, ss64, ysb, (m0 + h0) // 128)
            if si == 0 and pending is not None:
                pb, pysb, pss = pending
                vlad = vlad_pool.tile([K, D + 2], F32, tag="vlad")
                ve = nc.gpsimd if POOL_TT else nc.vector
                emit_pass(pb, pysb, pss, 0, SUB, ve, vlad)
                emit_finalize(pb, vlad, ve)
                pending = None
            if (b == last and si == 3 and CHUNK_LAST
                    and MODE != "one_span"):
                vlad_last = vlad_pool.tile([K, D + 2], F32, tag="vlad")
                ve = nc.gpsimd if POOL_TT else nc.vector
                emit_pass(b, ysb, ss64, 0, SUB // 2, ve, vlad_last)
        if b < last:
            pending = (b, ysb, ss64)
    if vlad_last is None:   # tail-only path (CHUNK_LAST off / one_span)
        vlad_last = vlad_pool.tile([K, D + 2], F32, tag="vlad")
        emit_pass(last, ysb, ss64, 0, SUB // 2,
                  nc.gpsimd if POOL_TT else nc.vector, vlad_last)
    emit_pass(last, ysb, ss64, SUB // 2, SUB, nc.vector, vlad_last)
    emit_finalize(last, vlad_last, nc.vector)

    nc.sync.dma_start(out_d.rearrange("b (k d) -> k b d", k=K), outsb[:])


_CACHE = {}


def _patch_act_tables():
    """Force all Exp/Ln/Square activations to resolve in the one table set
    that contains them all (natural_log_exp_and_others), so bacc's
    insert_act_table_loads emits a single hoisted LoadActFuncSet instead of
    thrashing between exp_and_others and natural_log per tile (~2.7us per
    reload).  List order/length is preserved so act_func_set_id stays a
    valid index into act_info.json."""
    import concourse.bacc as bacc_mod
    import concourse.hw_specs as hw_specs
    if _CACHE.get("act_patched"):
        return
    orig = hw_specs.get_activation_tables
    AF = mybir.ActivationFunctionType
    strip = {AF.Exp, AF.Ln, AF.Square, AF.Copy, AF.Identity}
    keep = "natural_log_exp_and_others"

    def patched(arch):
        tables = orig(arch)
        return {
            name: (set(fns) if name == keep else set(fns) - strip)
            for name, fns in tables.items()
        }

    bacc_mod.get_activation_tables = patched
    _CACHE["act_patched"] = True


def _declare_io(nc):
    xt_d = nc.dram_tensor("xt", [B_LOC, C, M], FP8,
                          kind="ExternalInput").ap()
    wcat_d = nc.dram_tensor("wcat", [NCH, 128, W], FP8,
                            kind="ExternalInput").ap()
    wlbc_d = nc.dram_tensor("wlbc", [128, 9], F32, kind="ExternalInput").ap()
    wl16_d = nc.dram_tensor("wl16", [128, 16], BF16,
                            kind="ExternalInput").ap()
    cb_d = nc.dram_tensor("cb", [K, 2, D], F32, kind="ExternalInput").ap()
    out_d = nc.dram_tensor("out", [B_LOC, K * D], F32, kind="ExternalOutput").ap()
    return out_d, xt_d, wcat_d, wlbc_d, wl16_d, cb_d


def _build_program():
    if "nc" in _CACHE:
        return _CACHE["nc"]
    _patch_act_tables()
    nc = bacc.Bacc("TRN2", target_bir_lowering=False, debug=False,
                   num_devices=N_CORES)
    io = _declare_io(nc)

    with tile.TileContext(nc) as tc:
        _netvlad_kernel(tc, *io)
    nc.compile()
    _CACHE["nc"] = nc
    return nc


def _prep_inputs(x, W_red, b_red, W_lin, b_lin, centroids):
    # fused weight: [r2 | ip2 | y0] columns
    wcat = np.concatenate([
        W_red.T @ W_lin.T,                              # [512, 8]
        (2.0 * IPS) * (W_red.T @ b_red)[:, None],       # [512, 1]
        W_red.T,                                        # [512, 64]
    ], axis=1)
    wcat = np.ascontiguousarray(wcat.astype(fp8).reshape(NCH, 128, W))
    wlbc = np.zeros((128, 9), np.float32)
    wlbc[:, 0:8] = W_lin @ b_red
    wlbc[:, 8] = float(b_red @ b_red)
    wl16 = np.zeros((128, 16), bf16)
    wl16[:, 0:8] = (W_lin @ b_red).astype(bf16)
    wl16[:, 8:16] = np.exp(b_lin).astype(bf16)
    cb = np.zeros((K, 2, D), np.float32)
    cb[:, 0, :] = -centroids            # negated: finalize chain is shorter
    cb[:, 1, :] = b_red[None, :]
    xt = np.ascontiguousarray(x.astype(fp8).transpose(0, 2, 1))     # [B, C, M]
    return {"xt": xt, "wcat": wcat, "wlbc": wlbc, "wl16": wl16, "cb": cb}


def kernel(x, mask, W_red, b_red, W_lin, b_lin, centroids, **kwargs):
    x = np.asarray(x, dtype=np.float32)
    W_red = np.asarray(W_red, dtype=np.float32)
    b_red = np.asarray(b_red, dtype=np.float32)
    W_lin = np.asarray(W_lin, dtype=np.float32)
    b_lin = np.asarray(b_lin, dtype=np.float32)
    centroids = np.asarray(centroids, dtype=np.float32)

    prep = _prep_inputs(x, W_red, b_red, W_lin, b_lin, centroids)
    xt = prep.pop("xt")

    nc = _build_program()
    in_maps = []
    for i in range(N_CORES):
        in_maps.append({
            "xt": np.ascontiguousarray(xt[i * B_LOC:(i + 1) * B_LOC]),
            **prep,
        })
    res = run_bass_kernel_spmd(nc, in_maps, list(range(N_CORES)),
                               **kwargs.get("_run_kwargs", {}))
    out = np.concatenate([res.results[i]["out"] for i in range(N_CORES)], axis=0)
    if kwargs.get("_return_raw"):
        return out, res
    return out


# revision 20
# speedup vs baseline: 1.0938x; 1.0938x over previous
"""NetVLAD pooling kernel for Trainium2, data-parallel over batch across 8 cores.

Computation per batch b (reference semantics):
  y      = x @ W_red.T + b_red            # [m, 64]
  yn     = y / ||y||_row                  # L2 normalize rows
  logits = yn @ W_lin.T + b_lin           # [m, 8]
  a      = softmax(logits, axis=1)
  vlad   = a.T @ yn - centroids * a.sum(0)[:, None]
  out    = l2norm_global(l2norm_rows(vlad).flatten())

Device-side structure (v14):
  - x ships pre-transposed [b, C, m] fp8e4m3; DMA at 2048-column granularity
    on the SP queue (consts go on the gpsimd queue so x loads start first).
  - Main matmuls run fp8 Normal mode (FWL active: 128-col stationary, fp8);
    4 matmuls per 128-row subtile.  DoubleRow measured slower (FD=73 < 128
    disables the FWL win; 95.1us -> 83.2us switching to Normal).
  - No bias matmul.  With y0 = x @ W_red.T (un-biased):
      yz[m, 0:8]  = r2  = y0 @ W_lin.T
      yz[m, 8]    = ip2 = 2*IPS * y0 . b_red   (extra fused weight column)
      yz[m, 9:73] = y0
    ss = ||y0+b||^2 = sum(y0^2) + ip2/IPS + ||b||^2 (Ln bias), logits bias
    W_lin@b_red added per-batch, and the aggregated b_red contribution
    restored at finalize via the ones-column.
  - Per tile: ACT stages all 73 cols PSUM->SBUF (one op); DVE squares the
    staged bf16 y0 at 2x and row-reduces into ss64.
  - Per batch, the softmax chain (batched over subtiles) runs on the Pool
    engine (TT/TS ops, SBUF-only) + ACT (Ln/Exp) + DVE (reduce/recip),
    with bf16 intermediates so DVE TT ops hit 2x mode.
    exp(b_lin) per-k factor cancels in the per-cluster L2 normalization
    denominator; it's still applied (am) so the softmax denominator is right.
  - vlad[k, 0:64 | asum | s] accumulated in PSUM over 64 small matmuls
    (stationary atil [128, 8] bf16); finalize per batch with negated
    centroids folded so the chain is short.
  - The LAST batch's pass runs in two chunks: subtiles 0:32 early (Pool,
    overlapped with remaining DMA), 32:64 + finalize in the tail on DVE/ACT.
"""
import numpy as np
import ml_dtypes
from contextlib import ExitStack

import concourse.bass as bass
import concourse.tile as tile
import concourse.bass_isa as bass_isa
from concourse import bacc, mybir
from concourse._compat import with_exitstack
from concourse.bass_utils import run_bass_kernel_spmd

bf16 = ml_dtypes.bfloat16
F32 = mybir.dt.float32
BF16 = mybir.dt.bfloat16
FP8 = mybir.dt.float8e4
fp8 = ml_dtypes.float8_e4m3

N_CORES = 8
B, M, C = 32, 8192, 512
K, D = 8, 64
B_LOC = B // N_CORES          # 4 batches per core
M_TILE = 1024                 # compute tile (rows)
DMA_COLS = 2048               # DMA granularity (2KB descriptors)
SUB = M // 128                # 64 subtiles of 128 rows per batch
NCH = C // 128                # 4 contraction chunks
W = 73                        # fused output cols: [r2(8) | ip2(1) | y0(64)]
W2 = 75                       # ysb cols: [r2 | ip2 | y0 | n | 1]
IPS = 64.0                    # ip2 column pre-scale (keeps fp8 weights normal)

XT_BUFS = 4
SB_BUFS = 2

# calibration knob (used by calib.py only; kernel() always runs "full"):
#   "full"      — normal kernel
#   "dma_only"  — emit only the DMA loads (no compute)
#   "one_span"  — DMA only span 0 per batch, compute reads it repeatedly
MODE = "full"
POOL_TT = True       # softmax-chain TT ops on Pool (False -> DVE)
POOL_MEMSET = True   # ones-column memset on Pool (False -> DVE)
USE_TTR = False      # tensor_tensor_reduce compiles but wedges the exec unit
                     # on this runtime (NRT_EXEC_UNIT_UNRECOVERABLE) - keep off
CHUNK_LAST = True    # split last batch's pass (False -> single tail pass)
STAGE_V13 = False    # True -> v13 staging (ACT 40-col copy + PSUM square,
                     # DVE 33-col copy + reduce)


@with_exitstack
def _netvlad_kernel(ctx: ExitStack, tc: tile.TileContext, out_d, xt_d, wcat_d,
                    wlbc_d, wl16_d, cb_d):
    nc = tc.nc
    AF = mybir.ActivationFunctionType
    OP = mybir.AluOpType

    consts = ctx.enter_context(tc.tile_pool(name="consts", bufs=1))
    xt_pool = ctx.enter_context(tc.tile_pool(name="xt", bufs=XT_BUFS))
    sb = ctx.enter_context(tc.tile_pool(name="work", bufs=SB_BUFS))
    bsb = ctx.enter_context(tc.tile_pool(name="bwork", bufs=SB_BUFS))
    outp = ctx.enter_context(tc.tile_pool(name="outp", bufs=1))
    yz_pool = ctx.enter_context(tc.tile_pool(name="yz", bufs=3, space="PSUM"))
    vlad_pool = ctx.enter_context(tc.tile_pool(name="vlad", bufs=2, space="PSUM"))

    if MODE == "dma_only":
        for b in range(B_LOC):
            xt_b = xt_d[b].rearrange("(j p) m -> p j m", p=128)
            for m0 in range(0, M, DMA_COLS):
                xt2 = xt_pool.tile([128, NCH, DMA_COLS], FP8, tag="xt")
                nc.sync.dma_start(xt2[:], xt_b[:, :, m0:m0 + DMA_COLS])
        return

    # wcat on the SP queue (SWDGE mis-handles its 512 tiny descriptors);
    # other consts on the gpsimd queue so xt loads queue right behind wcat
    wcat = consts.tile([128, NCH, W], FP8)
    nc.sync.dma_start(wcat[:], wcat_d.rearrange("j p t -> p j t"))
    wlbc = consts.tile([128, 9], F32)     # [W_lin@b_red (8, unused) | ||b||^2]
    nc.gpsimd.dma_start(wlbc[:], wlbc_d[:])
    wl16 = consts.tile([128, 16], BF16)   # [W_lin@b_red (8) | exp(b_lin) (8)]
    nc.gpsimd.dma_start(wl16[:], wl16_d[:])
    cb = consts.tile([K, 2, D], F32)      # [-centroids | b_red broadcast]
    nc.gpsimd.dma_start(cb[:], cb_d[:])

    outsb = outp.tile([K, B_LOC, D], F32)

    def process_tile(xt2, h0, rows, ss64, ysb, sub0):
        """One compute tile of `rows` rows starting at col h0 of xt2."""
        sub = rows // 128
        yz = yz_pool.tile([128, sub, 128], F32, tag="yz")
        for s in range(sub):
            for j in range(4):
                nc.tensor.matmul(
                    yz[:, s, :W],
                    xt2[:, j, h0 + s * 128:h0 + (s + 1) * 128],
                    wcat[:, j, :],
                    start=(j == 0), stop=(j == 3),
                )
        if STAGE_V13:
            # v13 split: ACT copies 40 cols + squares from PSUM; DVE copies 33
            nc.scalar.activation(ysb[:, sub0:sub0 + sub, 33:W],
                                 yz[:, :, 33:W], AF.Copy)
            nc.vector.tensor_copy(ysb[:, sub0:sub0 + sub, 0:33],
                                  yz[:, :, 0:33])
            sqs = sb.tile([128, 8, D], BF16, tag="sqs")
            nc.scalar.activation(sqs[:, :sub, :], yz[:, :, 9:W], AF.Square)
            nc.vector.reduce_sum(ss64[:, sub0:sub0 + sub], sqs[:, :sub, :],
                                 axis=mybir.AxisListType.X)
            return
        # stage all 73 cols PSUM->SBUF in one ACT op
        nc.scalar.activation(ysb[:, sub0:sub0 + sub, 0:W], yz[:, :, 0:W],
                             AF.Copy)
        # DVE squares the staged bf16 y0 (2x mode) and row-reduces into ss64
        sqs = sb.tile([128, 8, D], BF16, tag="sqs")
        nc.vector.tensor_tensor(
            out=sqs[:, :sub, :], in0=ysb[:, sub0:sub0 + sub, 9:W],
            in1=ysb[:, sub0:sub0 + sub, 9:W], op=OP.mult)
        nc.vector.reduce_sum(ss64[:, sub0:sub0 + sub], sqs[:, :sub, :],
                             axis=mybir.AxisListType.X)

    def emit_pass(b, ysb, ss64, s0, s1, ve, vlad):
        """Softmax-weight chain + vlad aggregation for subtiles [s0, s1).

        ve: engine namespace for the TT/TS ops (nc.gpsimd when overlapped
        under later DMA, nc.vector for the tail chunk)."""
        ns = s1 - s0
        # ss = sum(y0^2) + ip2/IPS (+ ||b||^2 via Ln bias)
        # (TensorScalarPtr is not a valid Pool opcode -> always DVE)
        sst = bsb.tile([128, ns], F32, tag=f"sst{s0}")
        nc.vector.scalar_tensor_tensor(
            out=sst[:], in0=ysb[:, s0:s1, 8], scalar=float(1.0 / IPS),
            in1=ss64[:, s0:s1], op0=OP.mult, op1=OP.add)
        lss = bsb.tile([128, ns], F32, tag=f"lss{s0}")
        nc.scalar.activation(lss[:], sst[:], AF.Ln, bias=wlbc[:, 8:9])
        inv = bsb.tile([128, ns], BF16, tag=f"inv{s0}")
        nc.scalar.activation(inv[:], lss[:], AF.Exp, scale=-0.5)
        # n = ||y|| = exp(+.5 ln ss) -> ysb col 73 (off critical path)
        nc.scalar.activation(ysb[:, s0:s1, W:W + 1], lss[:].unsqueeze(2),
                             AF.Exp, scale=0.5)
        invb = inv[:].unsqueeze(2).broadcast_to([128, ns, K])
        # t64 = (r2 + W_lin@b_red) * inv  (bf16 throughout)
        t0 = bsb.tile([128, ns, K], BF16, tag=f"t0{s0}")
        ve.tensor_tensor(
            out=t0[:], in0=ysb[:, s0:s1, 0:K],
            in1=wl16[:, 0:8].unsqueeze(1).broadcast_to([128, ns, K]),
            op=OP.add)
        t64 = bsb.tile([128, ns, K], BF16, tag=f"t64{s0}")
        ve.tensor_tensor(out=t64[:], in0=t0[:], in1=invb, op=OP.mult)
        e64 = bsb.tile([128, ns, K], BF16, tag=f"e64{s0}")
        nc.scalar.activation(e64[:], t64[:], AF.Exp)
        am = bsb.tile([128, ns, K], BF16, tag=f"am{s0}")
        ve.tensor_tensor(
            out=am[:], in0=e64[:],
            in1=wl16[:, 8:16].unsqueeze(1).broadcast_to([128, ns, K]),
            op=OP.mult)
        rs = bsb.tile([128, ns], F32, tag=f"rs{s0}")
        nc.vector.reduce_sum(rs[:], am[:], axis=mybir.AxisListType.X)
        rr = bsb.tile([128, ns], F32, tag=f"rr{s0}")
        nc.vector.reciprocal(rr[:], rs[:])
        q = bsb.tile([128, ns], BF16, tag=f"q{s0}")
        nc.vector.tensor_tensor(out=q[:], in0=inv[:], in1=rr[:], op=OP.mult)
        atil = bsb.tile([128, ns, K], BF16, tag=f"atil{s0}")
        ve.tensor_tensor(out=atil[:], in0=am[:],
                         in1=q[:].unsqueeze(2).broadcast_to([128, ns, K]),
                         op=OP.mult)
        # vlad[k] = [sum_m atil*y0 | sum atil*n (=a.sum) | sum atil (=s_k)]
        for s in range(s0, s1):
            nc.tensor.matmul(
                vlad[:], atil[:, s - s0, :], ysb[:, s, 9:W2],
                start=(s == 0), stop=(s == SUB - 1),
            )

    def emit_finalize(b, vlad, ve):
        """v = vlad_y + b*s_k - cent*asum, intra-normalize, global normalize.
        (cb[:,0] holds NEGATED centroids so the chain is short.)"""
        vz = sb.tile([K, D + 2], F32, tag="vz")
        nc.scalar.activation(vz[:], vlad[:], AF.Copy)
        bs = sb.tile([K, D], F32, tag="bs")
        nc.vector.tensor_scalar_mul(bs[:], cb[:, 1, :], vz[:, D + 1:D + 2])
        w = sb.tile([K, D], F32, tag="w")
        nc.vector.scalar_tensor_tensor(
            out=w[:], in0=cb[:, 0, :], scalar=vz[:, D:D + 1],
            in1=bs[:], op0=OP.mult, op1=OP.add)
        v = sb.tile([K, D], F32, tag="v")
        ve.tensor_tensor(out=v[:], in0=vz[:, 0:D], in1=w[:], op=OP.add)
        sck = sb.tile([K, D], F32, tag="sck")
        ssk = sb.tile([K, 1], F32, tag="ssk")
        if USE_TTR:
            nc.vector.tensor_tensor_reduce(
                out=sck[:], in0=v[:], in1=v[:], scale=1.0, scalar=0.0,
                op0=OP.mult, op1=OP.add, accum_out=ssk[:])
        else:
            nc.vector.tensor_tensor(out=sck[:], in0=v[:], in1=v[:],
                                    op=OP.mult)
            nc.vector.reduce_sum(ssk[:], sck[:], axis=mybir.AxisListType.X)
        lk = sb.tile([K, 1], F32, tag="lk")
        nc.scalar.activation(lk[:], ssk[:], AF.Ln)
        invk = sb.tile([K, 1], F32, tag="invk")
        nc.scalar.activation(invk[:], lk[:], AF.Exp, scale=-0.5)
        # after intra-normalization each of the K rows has norm exactly 1,
        # so the global norm is sqrt(K); fold 1/sqrt(K) into the multiply
        nc.vector.tensor_scalar(
            out=outsb[:, b, :], in0=v[:], scalar1=invk[:],
            scalar2=float(1.0 / np.sqrt(K)), op0=OP.mult, op1=OP.mult)

    # software pipeline: batch b-1's deferred pass is emitted after batch b's
    # first DMA span so its serial chain overlaps batch b's main matmuls.
    # The last batch's pass is chunked: first half early (Pool), second half
    # + finalize in the tail (DVE).
    pending = None
    vlad_last = None
    last = B_LOC - 1
    for b in range(B_LOC):
        ysb = bsb.tile([128, SUB, W2], BF16, tag="ysb")
        ss64 = bsb.tile([128, SUB], F32, tag="ss64")
        # ones column (cheap; per-batch so every read has a tracked writer)
        me = nc.gpsimd if POOL_MEMSET else nc.vector
        me.memset(ysb[:, :, W + 1:W + 2], 1.0)
        xt_b = xt_d[b].rearrange("(j p) m -> p j m", p=128)

        # DMA spans; batch 0 starts finer so the pipeline fills early
        if b == 0 and MODE != "one_span":
            spans = [(0, 512), (512, 512), (1024, 1024), (2048, 2048),
                     (4096, 2048), (6144, 2048)]
        else:
            spans = [(m0, DMA_COLS) for m0 in range(0, M, DMA_COLS)]
        first_xt = None
        for si, (m0, cols) in enumerate(spans):
            if MODE == "one_span" and first_xt is not None:
                xt2 = first_xt
            else:
                xt2 = xt_pool.tile([128, NCH, cols], FP8, tag="xt")
                nc.sync.dma_start(xt2[:], xt_b[:, :, m0:m0 + cols])
                if first_xt is None:
                    first_xt = xt2
            for h0 in range(0, cols, M_TILE):
                rows = min(M_TILE, cols - h0)
                process_tile(xt2, h0, rows, ss64, ysb, (m0 + h0) // 128)
            if si == 0 and pending is not None:
                pb, pysb, pss = pending
                vlad = vlad_pool.tile([K, D + 2], F32, tag="vlad")
                ve = nc.gpsimd if POOL_TT else nc.vector
                emit_pass(pb, pysb, pss, 0, SUB, ve, vlad)
                emit_finalize(pb, vlad, ve)
                pending = None
            if (b == last and si == 3 and CHUNK_LAST
                    and MODE != "one_span"):
                vlad_last = vlad_pool.tile([K, D + 2], F32, tag="vlad")
                ve = nc.gpsimd if POOL_TT else nc.vector
                emit_pass(b, ysb, ss64, 0, SUB // 2, ve, vlad_last)
        if b < last:
            pending = (b, ysb, ss64)
    if vlad_last is None:   # tail-only path (CHUNK_LAST off / one_span)
        vlad_last = vlad_pool.tile([K, D + 2], F32, tag="vlad")
        emit_pass(last, ysb, ss64, 0, SUB // 2,
                  nc.gpsimd if POOL_TT else nc.vector, vlad_last)
    emit_pass(last, ysb, ss64, SUB // 2, SUB, nc.vector, vlad_last)
    emit_finalize(last, vlad_last, nc.vector)

    nc.sync.dma_start(out_d.rearrange("b (k d) -> k b d", k=K), outsb[:])


_CACHE = {}


def _patch_act_tables():
    """Force all Exp/Ln/Square activations to resolve in the one table set
    that contains them all (natural_log_exp_and_others), so bacc's
    insert_act_table_loads emits a single hoisted LoadActFuncSet instead of
    thrashing between exp_and_others and natural_log per tile (~2.7us per
    reload).  List order/length is preserved so act_func_set_id stays a
    valid index into act_info.json."""
    import concourse.bacc as bacc_mod
    import concourse.hw_specs as hw_specs
    if _CACHE.get("act_patched"):
        return
    orig = hw_specs.get_activation_tables
    AF = mybir.ActivationFunctionType
    strip = {AF.Exp, AF.Ln, AF.Square, AF.Copy, AF.Identity}
    keep = "natural_log_exp_and_others"

    def patched(arch):
        tables = orig(arch)
        return {
            name: (set(fns) if name == keep else set(fns) - strip)
            for name, fns in tables.items()
        }

    bacc_mod.get_activation_tables = patched
    _CACHE["act_patched"] = True


def _declare_io(nc):
    xt_d = nc.dram_tensor("xt", [B_LOC, C, M], FP8,
                          kind="ExternalInput").ap()
    wcat_d = nc.dram_tensor("wcat", [NCH, 128, W], FP8,
                            kind="ExternalInput").ap()
    wlbc_d = nc.dram_tensor("wlbc", [128, 9], F32, kind="ExternalInput").ap()
    wl16_d = nc.dram_tensor("wl16", [128, 16], BF16,
                            kind="ExternalInput").ap()
    cb_d = nc.dram_tensor("cb", [K, 2, D], F32, kind="ExternalInput").ap()
    out_d = nc.dram_tensor("out", [B_LOC, K * D], F32, kind="ExternalOutput").ap()
    return out_d, xt_d, wcat_d, wlbc_d, wl16_d, cb_d


def _build_program():
    if "nc" in _CACHE:
        return _CACHE["nc"]
    _patch_act_tables()
    nc = bacc.Bacc("TRN2", target_bir_lowering=False, debug=False,
                   num_devices=N_CORES)
    io = _declare_io(nc)

    with tile.TileContext(nc) as tc:
        _netvlad_kernel(tc, *io)
    nc.compile()
    _CACHE["nc"] = nc
    return nc


def _prep_inputs(x, W_red, b_red, W_lin, b_lin, centroids):
    # fused weight: [r2 | ip2 | y0] columns
    wcat = np.concatenate([
        W_red.T @ W_lin.T,                              # [512, 8]
        (2.0 * IPS) * (W_red.T @ b_red)[:, None],       # [512, 1]
        W_red.T,                                        # [512, 64]
    ], axis=1)
    wcat = np.ascontiguousarray(wcat.astype(fp8).reshape(NCH, 128, W))
    wlbc = np.zeros((128, 9), np.float32)
    wlbc[:, 0:8] = W_lin @ b_red
    wlbc[:, 8] = float(b_red @ b_red)
    wl16 = np.zeros((128, 16), bf16)
    wl16[:, 0:8] = (W_lin @ b_red).astype(bf16)
    wl16[:, 8:16] = np.exp(b_lin).astype(bf16)
    cb = np.zeros((K, 2, D), np.float32)
    cb[:, 0, :] = -centroids            # negated: finalize chain is shorter
    cb[:, 1, :] = b_red[None, :]
    xt = np.ascontiguousarray(x.astype(fp8).transpose(0, 2, 1))     # [B, C, M]
    return {"xt": xt, "wcat": wcat, "wlbc": wlbc, "wl16": wl16, "cb": cb}


def kernel(x, mask, W_red, b_red, W_lin, b_lin, centroids, **kwargs):
    x = np.asarray(x, dtype=np.float32)
    W_red = np.asarray(W_red, dtype=np.float32)
    b_red = np.asarray(b_red, dtype=np.float32)
    W_lin = np.asarray(W_lin, dtype=np.float32)
    b_lin = np.asarray(b_lin, dtype=np.float32)
    centroids = np.asarray(centroids, dtype=np.float32)

    prep = _prep_inputs(x, W_red, b_red, W_lin, b_lin, centroids)
    xt = prep.pop("xt")

    nc = _build_program()
    in_maps = []
    for i in range(N_CORES):
        in_maps.append({
            "xt": np.ascontiguousarray(xt[i * B_LOC:(i + 1) * B_LOC]),
            **prep,
        })
    res = run_bass_kernel_spmd(nc, in_maps, list(range(N_CORES)),
                               **kwargs.get("_run_kwargs", {}))
    out = np.concatenate([res.results[i]["out"] for i in range(N_CORES)], axis=0)
    if kwargs.get("_return_raw"):
        return out, res
    return out
